# revision 4
# baseline (speedup 1.0000x reference)
"""Trainium2 Bass kernel for the DART masked-MLP + log-semiring chain model.

Computes, for B=8192 samples distributed over 8 NeuronCores (1024 each):
  h1 = relu(x @ (m0*W0).T + b0)
  h2 = relu(h1 @ (m1*W1).T + b1)
  h3 = relu(h2 @ (m2*W2).T + b2)
  theta = (h3 @ (m3*W3).T + b3) -> (B, 128, 2, 4, 4) = (mu, alpha)
  logp  = -0.5*((x - mu)*exp(-alpha))**2 - alpha - 0.5*log(2pi) - log(4)
  out   = logexpmm(first, logexpmm(chain(inner), last))   # (B, 1, 1)

Device strategy (per core):
  - MADE masks are premultiplied into the weights host-side; hidden units are
    sorted by MADE degree so the masked weight matrices become block lower
    triangular and ~47% of the K-chunks can be skipped entirely.
  - Matmuls run in bf16 (fp32 PSUM accumulation).  Activations are kept in
    transposed layout [hidden, batch] so the contraction dim always sits on
    partitions.  The final layer flips orientation (stationary = activations)
    to produce theta as [batch, 4096] so the per-sample chain can run with
    batch on partitions.
  - b3 is added via a K=1 matmul against a ones row vector.
  - The 126-step log-semiring chain runs in the linear domain:
    P = exp(logp) precomputed for all positions, u' = P^T u per step with a
    max-renormalization every 4 steps (accumulated in log space).
"""

import math

import numpy as np
import ml_dtypes

I = 128          # input size / positions
H = 2048         # hidden
A = 4            # alpha_dim
K = 2 * A * A    # 32 theta entries per position
B = 8192
NCORES = 8
BL = B // NCORES          # 1024 samples per core
NG = BL // 128            # 8 sample groups of 128
NK = H // 128             # 16 hidden chunks
NQ = (I * K) // 512       # 8 output q-chunks (512 wide = 16 positions)
C0 = 0.5 * math.log(2.0 * math.pi) + math.log(4.0)

_bf16 = ml_dtypes.bfloat16


def _make_meta():
    """Degree sort + triangular chunk metadata (static)."""
    hdeg = np.arange(H) % (I - 1)
    perm = np.argsort(hdeg, kind="stable")
    sdeg = hdeg[perm]
    # L2/L3: out-chunk m needs in-chunks k whose min degree <= max out degree
    km_l = []
    for m in range(NK):
        dhi = sdeg[128 * m + 127]
        km_l.append(max(k for k in range(NK) if sdeg[128 * k] <= dhi))
    # L4: q-chunk qc covers positions 16qc..16qc+15, out degree = pos-1
    km_4 = []
    for qc in range(NQ):
        dhi = 16 * qc + 15 - 1
        cands = [k for k in range(NK) if sdeg[128 * k] <= dhi]
        km_4.append(max(cands) if cands else -1)
    return perm, km_l, km_4


_PERM, _KM_L, _KM_4 = _make_meta()
_OFF1 = np.cumsum([0] + [(k + 1) * 128 for k in _KM_L]).tolist()
_OFF3 = np.cumsum([0] + [(k + 1) * 512 for k in _KM_4]).tolist()


def _prep_inputs(x, W0, b0, W1, b1, W2, b2, W3, b3):
    """Host-side: premask, degree-sort, pack and cast the weights."""
    inp = np.arange(I)
    degrees = [inp] + [np.arange(H) % (I - 1) for _ in range(3)] + [np.arange(I) - 1]
    masks = [
        (d1[:, None] >= d0[None, :]).astype(np.float32)
        for d0, d1 in zip(degrees[:-1], degrees[1:])
    ]
    masks[-1] = np.repeat(masks[-1], K, axis=0)

    p = _PERM
    W0s = (masks[0] * W0)[p]
    b0s = b0[p]
    W1s = (masks[1] * W1)[p][:, p]
    b1s = b1[p]
    W2s = (masks[2] * W2)[p][:, p]
    b2s = b2[p]
    W3s = (masks[3] * W3)[:, p]

    w0t = np.ascontiguousarray(W0s.T).astype(_bf16)           # [128, 2048]

    def pack_hidden(Ws):
        WT = Ws.T  # [in, out]
        cols = []
        for m in range(NK):
            for k in range(_KM_L[m] + 1):
                cols.append(WT[128 * k:128 * (k + 1), 128 * m:128 * (m + 1)])
        return np.ascontiguousarray(np.concatenate(cols, axis=1)).astype(_bf16)

    w1t = pack_hidden(W1s)
    w2t = pack_hidden(W2s)

    W3T = W3s.T  # [2048, 4096]
    cols = []
    for qc in range(NQ):
        for k in range(_KM_4[qc] + 1):
            cols.append(W3T[128 * k:128 * (k + 1), 512 * qc:512 * (qc + 1)])
    w3t = np.ascontiguousarray(np.concatenate(cols, axis=1)).astype(_bf16)

    b0r = np.ascontiguousarray(b0s.reshape(NK, 128).T).astype(np.float32)
    b1r = np.ascontiguousarray(b1s.reshape(NK, 128).T).astype(np.float32)
    b2r = np.ascontiguousarray(b2s.reshape(NK, 128).T).astype(np.float32)
    b3r = np.ascontiguousarray(b3[None, :]).astype(_bf16)     # [1, 4096]

    common = dict(w0t=w0t, w1t=w1t, w2t=w2t, w3t=w3t,
                  b0r=b0r, b1r=b1r, b2r=b2r, b3r=b3r)
    in_maps = []
    for c in range(NCORES):
        m = dict(common)
        m["x"] = np.ascontiguousarray(x[c * BL:(c + 1) * BL]).astype(np.float32)
        in_maps.append(m)
    return in_maps


_NC_CACHE = {}


def _build_nc():
    import concourse.bacc as bacc
    import concourse.tile as tile
    import concourse.mybir as mybir
    from concourse.masks import make_identity
    from contextlib import ExitStack

    f32 = mybir.dt.float32
    bf16 = mybir.dt.bfloat16
    AF = mybir.ActivationFunctionType
    ALU = mybir.AluOpType
    AX = mybir.AxisListType

    nc = bacc.Bacc("TRN2")
    x_d = nc.declare_dram_parameter("x", [BL, I], f32, isOutput=False)
    w0_d = nc.declare_dram_parameter("w0t", [I, H], bf16, isOutput=False)
    w1_d = nc.declare_dram_parameter("w1t", [128, _OFF1[-1]], bf16, isOutput=False)
    w2_d = nc.declare_dram_parameter("w2t", [128, _OFF1[-1]], bf16, isOutput=False)
    w3_d = nc.declare_dram_parameter("w3t", [128, _OFF3[-1]], bf16, isOutput=False)
    b0_d = nc.declare_dram_parameter("b0r", [128, NK], f32, isOutput=False)
    b1_d = nc.declare_dram_parameter("b1r", [128, NK], f32, isOutput=False)
    b2_d = nc.declare_dram_parameter("b2r", [128, NK], f32, isOutput=False)
    b3_d = nc.declare_dram_parameter("b3r", [1, I * K], bf16, isOutput=False)
    out_d = nc.declare_dram_parameter("out", [128, NG], f32, isOutput=True)

    with ExitStack() as ctx:
        tc = ctx.enter_context(tile.TileContext(nc))
        consts = ctx.enter_context(tc.tile_pool(name="consts", bufs=1))
        pspool = ctx.enter_context(tc.tile_pool(name="psl", bufs=2, space="PSUM"))
        ps4pool = ctx.enter_context(tc.tile_pool(name="ps4", bufs=4, space="PSUM"))
        a13p = ctx.enter_context(tc.tile_pool(name="a13p", bufs=NK))
        a2p = ctx.enter_context(tc.tile_pool(name="a2p", bufs=NK))
        wpool = ctx.enter_context(tc.tile_pool(name="wl", bufs=3))
        w3pool = ctx.enter_context(tc.tile_pool(name="w3", bufs=18))
        pallpool = ctx.enter_context(tc.tile_pool(name="pallp", bufs=NQ))
        ltmp = ctx.enter_context(tc.tile_pool(name="ltmp", bufs=3))
        chpool = ctx.enter_context(tc.tile_pool(name="ch", bufs=2))

        # ---- constants ----
        xf = consts.tile([128, NG, I], f32)         # x[p, g, i] = x[g*128+p, i]
        nc.sync.dma_start(out=xf, in_=x_d[:, :].rearrange("(g p) i -> p g i", p=128))
        w0sb = consts.tile([128, H], bf16)
        nc.sync.dma_start(out=w0sb, in_=w0_d[:, :])
        b0sb = consts.tile([128, NK], f32)
        nc.sync.dma_start(out=b0sb, in_=b0_d[:, :])
        b1sb = consts.tile([128, NK], f32)
        nc.sync.dma_start(out=b1sb, in_=b1_d[:, :])
        b2sb = consts.tile([128, NK], f32)
        nc.sync.dma_start(out=b2sb, in_=b2_d[:, :])
        b3sb = consts.tile([1, I * K], bf16)
        nc.sync.dma_start(out=b3sb, in_=b3_d[:, :])
        ones1 = consts.tile([1, 128], bf16)
        nc.vector.memset(ones1, 1.0)
        czero = consts.tile([128, 1], f32)
        nc.vector.memset(czero, 0.0)
        nc.const_aps.aps[(f32, 0.0)] = czero[:, :]
        cnegc = consts.tile([128, 1], f32)
        nc.vector.memset(cnegc, -C0)
        ident = consts.tile([128, 128], f32)
        make_identity(nc, ident[:, :])
        logs = consts.tile([128, NG, 32], f32)
        nc.vector.memset(logs, 0.0)
        xt = consts.tile([128, BL], bf16)           # xT[i, g*128+b]

        # ---- transpose x (PE transpose, cast to bf16 on copy-out) ----
        for g in range(NG):
            pst = pspool.tile([128, 128], f32, tag="psl", name=f"pst{g}")
            nc.tensor.transpose(pst[:, :], xf[:, g, :], ident[:, :])
            nc.scalar.copy(out=xt[:, g * 128:(g + 1) * 128], in_=pst[:, :])

        # ---- layer 1 ----
        A1 = []
        for m in range(NK):
            ps = pspool.tile([128, BL], f32, tag="psl", name=f"ps1_{m}")
            for n in range(2):
                nc.tensor.matmul(ps[:, n * 512:(n + 1) * 512],
                                 w0sb[:, m * 128:(m + 1) * 128],
                                 xt[:, n * 512:(n + 1) * 512],
                                 start=True, stop=True)
            a = a13p.tile([128, BL], bf16, tag="a13", name=f"a1_{m}")
            nc.scalar.activation(a[:, :], ps[:, :], AF.Relu,
                                 bias=b0sb[:, m:m + 1], scale=1.0)
            A1.append(a)

        # ---- layers 2 and 3 ----
        def hidden_layer(w_dram, Ain, bsb, pool, tagp):
            Aout = []
            for m in range(NK):
                km = _KM_L[m]
                wt = wpool.tile([128, (km + 1) * 128], bf16, tag="wl",
                                name=f"w{tagp}_{m}")
                nc.sync.dma_start(out=wt, in_=w_dram[:, _OFF1[m]:_OFF1[m + 1]])
                ps = pspool.tile([128, BL], f32, tag="psl", name=f"ps{tagp}_{m}")
                for n in range(2):
                    for k in range(km + 1):
                        nc.tensor.matmul(ps[:, n * 512:(n + 1) * 512],
                                         wt[:, k * 128:(k + 1) * 128],
                                         Ain[k][:, n * 512:(n + 1) * 512],
                                         start=(k == 0), stop=(k == km))
                a = pool.tile([128, BL], bf16, tag=tagp, name=f"a{tagp}_{m}")
                nc.scalar.activation(a[:, :], ps[:, :], AF.Relu,
                                     bias=bsb[:, m:m + 1], scale=1.0)
                Aout.append(a)
            return Aout

        A2 = hidden_layer(w1_d, A1, b1sb, a2p, "a2")
        A3 = hidden_layer(w2_d, A2, b2sb, a13p, "a13")

        # ---- layer 4 + logp + chain ----
        PALL = []
        for qc in range(NQ):
            pall = pallpool.tile([128, 16, NG, 16], f32, tag="pall",
                                 name=f"pall_{qc}")
            PALL.append(pall)

        state = {"u": None, "tpar": 0}

        def chain_step(t):
            qc, li = divmod(t, 16)
            tmp = chpool.tile([128, NG, 4, 4], f32, tag="tmp", name=f"tmp{t}")
            Pv = PALL[qc][:, li, :, :].rearrange("p g (k j) -> p g j k", k=4)
            ubc = state["u"][:, :, None, :].broadcast_to([128, NG, 4, 4])
            nc.vector.tensor_mul(tmp[:, :, :, :], ubc, Pv)
            unew = chpool.tile([128, NG, 4], f32, tag="u", name=f"u{t}")
            nc.vector.tensor_reduce(unew[:, :, :], tmp[:, :, :, :],
                                    axis=AX.X, op=ALU.add)
            state["u"] = unew
            if t % 4 == 0:
                idx = t // 4 - 1
                m8 = chpool.tile([128, NG], f32, tag="m8", name=f"m8_{t}")
                nc.vector.tensor_reduce(m8[:, :], unew[:, :, :],
                                        axis=AX.X, op=ALU.max)
                nc.scalar.activation(logs[:, :, idx], m8[:, :], AF.Ln)
                r8 = chpool.tile([128, NG], f32, tag="r8", name=f"r8_{t}")
                nc.vector.reciprocal(r8[:, :], m8[:, :])
                un = chpool.tile([128, NG, 4], f32, tag="u", name=f"un{t}")
                nc.vector.tensor_mul(un[:, :, :], unew[:, :, :],
                                     r8[:, :, None].broadcast_to([128, NG, 4]))
                state["u"] = un

        for qc in range(NQ):
            km = _KM_4[qc]
            wts = []
            for k in range(km + 1):
                w3t_ = w3pool.tile([128, 512], bf16, tag="w3", name=f"w3_{qc}_{k}")
                nc.sync.dma_start(
                    out=w3t_,
                    in_=w3_d[:, _OFF3[qc] + k * 512:_OFF3[qc] + (k + 1) * 512])
                wts.append(w3t_)
            for g in range(NG):
                ps = ps4pool.tile([128, 512], f32, tag="ps4", name=f"ps4_{qc}_{g}")
                for k in range(km + 1):
                    nc.tensor.matmul(ps[:, :], A3[k][:, g * 128:(g + 1) * 128],
                                     wts[k][:, :], start=(k == 0), stop=False)
                nc.tensor.matmul(ps[:, :], ones1[0:1, :],
                                 b3sb[0:1, qc * 512:(qc + 1) * 512],
                                 start=False, stop=True)
                psv = ps[:, :].rearrange("p (i t e) -> p i t e", t=2, e=16)
                mu_ap = psv[:, :, 0, :]
                al_ap = psv[:, :, 1, :]
                et = ltmp.tile([128, 16, 16], f32, tag="et", name=f"et{qc}_{g}")
                nc.scalar.activation(et[:, :, :], al_ap, AF.Exp, scale=-1.0)
                dt_ = ltmp.tile([128, 16, 16], f32, tag="dt", name=f"dt{qc}_{g}")
                xbc = xf[:, g, qc * 16:(qc + 1) * 16][:, :, None] \
                    .broadcast_to([128, 16, 16])
                nc.vector.tensor_sub(dt_[:, :, :], xbc, mu_ap)
                tt_ = ltmp.tile([128, 16, 16], f32, tag="tt", name=f"tt{qc}_{g}")
                nc.vector.tensor_mul(tt_[:, :, :], dt_[:, :, :], et[:, :, :])
                sq = ltmp.tile([128, 16, 16], f32, tag="sq", name=f"sq{qc}_{g}")
                nc.gpsimd.tensor_mul(sq[:, :, :], tt_[:, :, :], tt_[:, :, :])
                p1 = ltmp.tile([128, 16, 16], f32, tag="p1", name=f"p1{qc}_{g}")
                nc.scalar.activation(p1[:, :, :], sq[:, :, :], AF.Exp,
                                     scale=-0.5, bias=cnegc[:, :])
                nc.gpsimd.tensor_mul(PALL[qc][:, :, g, :], p1[:, :, :],
                                     et[:, :, :])

            # chain steps whose P block just completed
            if qc == 0:
                u0 = chpool.tile([128, NG, 4], f32, tag="u", name="u0")
                nc.vector.tensor_copy(u0[:, :, :], PALL[0][:, 0, :, 0:4])
                state["u"] = u0
                for t in range(1, 16):
                    chain_step(t)
            elif qc < NQ - 1:
                for t in range(16 * qc, 16 * (qc + 1)):
                    chain_step(t)
            else:
                for t in range(16 * qc, 16 * qc + 15):
                    chain_step(t)

        # ---- finalize ----
        tmp2 = chpool.tile([128, NG, 4], f32, tag="tmp2")
        Pl = PALL[NQ - 1][:, 15, :, :].rearrange("p g (k j) -> p g k j", k=4)[:, :, :, 0]
        nc.vector.tensor_mul(tmp2[:, :, :], state["u"][:, :, :], Pl)
        tot = chpool.tile([128, NG], f32, tag="tot")
        nc.vector.tensor_reduce(tot[:, :], tmp2[:, :, :], axis=AX.X, op=ALU.add)
        lgt = chpool.tile([128, NG], f32, tag="lgt")
        nc.scalar.activation(lgt[:, :], tot[:, :], AF.Ln)
        ssum = chpool.tile([128, NG], f32, tag="ssum")
        nc.vector.tensor_reduce(ssum[:, :], logs[:, :, :], axis=AX.X, op=ALU.add)
        res = chpool.tile([128, NG], f32, tag="res")
        nc.vector.tensor_add(res[:, :], lgt[:, :], ssum[:, :])
        nc.sync.dma_start(out=out_d[:, :], in_=res[:, :])

    nc.compile()
    return nc


def _get_nc():
    if "nc" not in _NC_CACHE:
        _NC_CACHE["nc"] = _build_nc()
    return _NC_CACHE["nc"]


def run_on_hw(in_maps, trace=False):
    from concourse.bass_utils import run_bass_kernel_spmd
    nc = _get_nc()
    return run_bass_kernel_spmd(nc, in_maps, list(range(NCORES)), trace=trace)


def kernel(**inputs):
    in_maps = _prep_inputs(
        inputs["x"], inputs["W0"], inputs["b0"], inputs["W1"], inputs["b1"],
        inputs["W2"], inputs["b2"], inputs["W3"], inputs["b3"])
    res = run_on_hw(in_maps)
    out = np.empty((B,), np.float32)
    for c in range(NCORES):
        out[c * BL:(c + 1) * BL] = res.results[c]["out"].T.reshape(BL)
    return out.reshape(B, 1, 1)
